# revision 9
# baseline (speedup 1.0000x reference)
# Trainium2 Bass kernel for nn_LinearNonlinearRelease.
#
# Pipeline (8 NeuronCores, time-sharded):
#   x --conv1(20-tap, per-cell, banded PE matmul)--> y --Sigmoid--> rp
#   rp --(PE-transpose re-layout)--> chunk-parallel scan layout
#   scan: 1024 time-chunks x 126 (cell,slot) lanes per core, 96 warmup +
#         128 main steps; smooth_clamp computed in median form with a
#         single Exp per clamp:
#            sc(x,H) = min(max(x,E), H-E),  E = min(exp(min(x,H-x)-1), 1)
#   rel --(transpose back)--> conv2(32-tap shared kernel) --affine--> out
import numpy as np

NUM_CELLS = 14
FREQ = 64
D = 1048576
STEADY = 10 * FREQ            # 640
K0 = 20
K1 = 32
PADDING = STEADY + (K0 - 1) + (K1 - 1)   # 690
T = D + PADDING - K0 + 1      # 1049247
NCORES = 8
CLEN = 128                    # scan chunk length (steps)
W_WARM = 128                  # warmup steps (full chunk, cap-init)


def _f32(x):
    return np.asarray(x, np.float32)


def _elu_np(x):
    return np.where(x > 0, x, np.expm1(x)).astype(np.float32)


def _smooth_clamp_np(x, high):
    x = _elu_np(np.float32(x) - np.float32(1.0)) + np.float32(1.0)
    x = _elu_np(np.float32(high) - np.float32(1.0) - x) - np.float32(high) + np.float32(1.0)
    return (-x).astype(np.float32)


def _compute_kernel_np(log_kernel_speed, cell_types):
    # replicate reference._compute_kernel in float32 numpy
    t = (np.float32(0.3) - np.arange(K0, dtype=np.float32) / np.float32(FREQ))[None, :]
    ks = np.exp(_f32(log_kernel_speed))[:, None].astype(np.float32)
    tau_r = (np.float32(0.05) * ks).astype(np.float32)
    tau_d = (np.float32(0.05) * ks).astype(np.float32)
    phi = (np.float32(-np.pi) * np.float32(0.2 / 1.4) * ks).astype(np.float32)
    kernel = (-(t / tau_r) ** 3 / (1.0 + t / tau_r)
              * np.exp(-((t / tau_d) ** 2))
              * np.cos(2.0 * np.float32(np.pi) * t / phi + np.float32(100.0))).astype(np.float32)
    kernel = kernel / np.linalg.norm(kernel.astype(np.float64), axis=1, keepdims=True).astype(np.float32)
    kernel = (-kernel * _f32(cell_types)[:, None]).astype(np.float32)
    return kernel  # (C, K0)


class _Prog:
    pass


_PROG_CACHE = {}


def build_program(SP):
    """Build the per-core Bass program. SP = per-core span (multiple of 16384)."""
    if SP in _PROG_CACHE:
        return _PROG_CACHE[SP]
    import concourse.bacc as bacc
    import concourse.mybir as mybir
    import concourse.tile as tile

    F32 = mybir.dt.float32
    F16 = mybir.dt.float16
    Alu = mybir.AluOpType
    Act = mybir.ActivationFunctionType

    NJj = SP // CLEN              # real span chunks (1024)
    NCH = NJj + 2                 # chunks incl pre-chunk and halo chunk
    FSLOT = (NCH + 127) // 128    # chunk-high slots per cell (9)
    NJ = FSLOT * 128              # tmaj col count (1152)
    FREE = NUM_CELLS * FSLOT      # scan free width (126)
    XS_LEN = NJ * 128             # per-core x slice length
    NCONV1 = NJj + 3              # conv1 cols needed (1027)
    U = SP // 128                 # coarse free per partition for output (1024)
    NSTEP = CLEN * FREE           # 16128 free elems in scan stream

    nc = bacc.Bacc(None, target_bir_lowering=False)

    xs_e = nc.declare_dram_parameter("xs", [XS_LEN], F32, isOutput=False)
    w1_e = nc.declare_dram_parameter("w1", [NUM_CELLS, 128, 128], F32, isOutput=False)
    w2_e = nc.declare_dram_parameter("w2", [NUM_CELLS, 128, 128], F32, isOutput=False)
    g1_e = nc.declare_dram_parameter("g1", [128, 128], F32, isOutput=False)
    g2_e = nc.declare_dram_parameter("g2", [128, 128], F32, isOutput=False)
    idf32_e = nc.declare_dram_parameter("idf32", [128, 128], F32, isOutput=False)
    idf16_e = nc.declare_dram_parameter("idf16", [128, 128], F16, isOutput=False)
    cc_e = nc.declare_dram_parameter("cc", [5, FREE], F32, isOutput=False)
    sg_e = nc.declare_dram_parameter("sg", [2, NUM_CELLS], F32, isOutput=False)
    fn_e = nc.declare_dram_parameter("fn", [2, NUM_CELLS], F32, isOutput=False)
    out_e = nc.declare_dram_parameter("out", [NUM_CELLS, SP], F32, isOutput=True)

    with tile.TileContext(nc) as tc:
        with tc.tile_pool(name="persist", bufs=1) as pp, \
             tc.tile_pool(name="wstage", bufs=2) as wp, \
             tc.tile_pool(name="tmaj", bufs=2) as mp, \
             tc.tile_pool(name="tmp", bufs=2) as sp, \
             tc.tile_pool(name="pconv", bufs=2, space="PSUM") as pcv, \
             tc.tile_pool(name="ptp", bufs=2, space="PSUM") as ptp:

            # ---- phase 0: loads & constants ----
            Xc = pp.tile([128, NJ], F32)
            nc.sync.dma_start(Xc[:], xs_e[:].rearrange("(p j) -> p j", j=NJ))
            idf32 = pp.tile([128, 128], F32)
            idf16 = pp.tile([128, 128], F16)
            nc.sync.dma_start(idf32[:], idf32_e[:])
            nc.sync.dma_start(idf16[:], idf16_e[:])
            g1 = pp.tile([128, 128], F32)
            g2 = pp.tile([128, 128], F32)
            nc.sync.dma_start(g1[:], g1_e[:])
            nc.sync.dma_start(g2[:], g2_e[:])
            # per-cell const tiles broadcast to all partitions
            HRT = pp.tile([128, FREE], F32)
            HIT = pp.tile([128, FREE], F32)
            CP12T = pp.tile([128, FREE], F32)
            CP01T = pp.tile([128, FREE], F32)
            HICPT = pp.tile([128, FREE], F32)
            for i, ct in enumerate([HRT, HIT, CP12T, CP01T, HICPT]):
                nc.sync.dma_start(ct[:], cc_e[i:i + 1, :].to_broadcast([128, FREE]))
            SGT = pp.tile([128, NUM_CELLS], F32)
            FNT = pp.tile([128, NUM_CELLS], F32)
            nc.sync.dma_start(SGT[:], sg_e[0:1, :].to_broadcast([128, NUM_CELLS]))
            nc.sync.dma_start(FNT[:], fn_e[0:1, :].to_broadcast([128, NUM_CELLS]))
            SBT = pp.tile([128, NUM_CELLS], F32)
            FBT = pp.tile([128, NUM_CELLS], F32)
            nc.sync.dma_start(SBT[:], sg_e[1:2, :].to_broadcast([128, NUM_CELLS]))
            nc.sync.dma_start(FBT[:], fn_e[1:2, :].to_broadcast([128, NUM_CELLS]))
            neg1 = pp.tile([128, 1], F32)
            nc.vector.memset(neg1[:], -1.0)

            # x fine-t layout: Xf[r, 9p+b] = Xc[p, 128b+r]
            Xf = pp.tile([128, NJ], F32)
            for b in range(FSLOT):
                tpp = ptp.tile([128, 128], F32, tag="tp")
                nc.tensor.transpose(tpp[:], Xc[:, b * 128:(b + 1) * 128], idf32[:])
                nc.vector.tensor_copy(Xf[:, b::FSLOT], tpp[:])

            rp_scan = pp.tile([128, NSTEP], F16)
            rel_scan = pp.tile([128, NSTEP], F16)

            # ---- phase 1: conv1 + sigmoid + re-layout, per cell ----
            col_blocks = []
            n0 = 0
            while n0 < NCONV1:
                nn = min(512, NCONV1 - n0)
                col_blocks.append((n0, nn))
                n0 += nn
            for c in range(NUM_CELLS):
                w1t = wp.tile([128, 128], F32, tag="w1t")
                w2t = wp.tile([128, 128], F32, tag="w2t")
                nc.sync.dma_start(w1t[:], w1_e[c])
                nc.sync.dma_start(w2t[:], w2_e[c])
                rpt = mp.tile([128, NJ], F16, tag="rpt")
                if NJ > NCONV1:
                    nc.gpsimd.memset(rpt[:, NCONV1:NJ], 0.5)
                for (b0, bn) in col_blocks:
                    ps = pcv.tile([128, 512], F32, tag="pconv")
                    nc.tensor.matmul(ps[:, 0:bn], w1t[:], Xf[:, b0:b0 + bn], start=True, stop=False)
                    nc.tensor.matmul(ps[:, 0:bn], w2t[:], Xf[:, b0 + 1:b0 + 1 + bn], start=False, stop=True)
                    nc.scalar.activation(rpt[:, b0:b0 + bn], ps[:, 0:bn], Act.Sigmoid,
                                         bias=SBT[:, c:c + 1], scale=SGT[:, c:c + 1])
                for f8 in range(FSLOT):
                    tpp = ptp.tile([128, 128], F16, tag="tp16")
                    nc.tensor.transpose(tpp[:], rpt[:, f8::FSLOT], idf16[:])
                    base = c * FSLOT + f8
                    nc.vector.tensor_copy(rp_scan[:, base:base + (CLEN - 1) * FREE + 1:FREE], tpp[:])

            # ---- phase 2: scan ----
            rrp = pp.tile([128, FREE], F32)
            ipt = pp.tile([128, FREE], F32)
            rrp2 = pp.tile([128, FREE], F32)
            ipt2 = pp.tile([128, FREE], F32)
            nc.vector.tensor_copy(rrp[:], HRT[:])
            nc.vector.tensor_copy(ipt[:], HIT[:])

            def step(s, rrp_t, ip_t, store):
                rp_s = rp_scan[:, s * FREE:(s + 1) * FREE]
                t1 = sp.tile([128, FREE], F32, tag="t1")
                rl = sp.tile([128, FREE], F32, tag="rl")
                m1 = sp.tile([128, FREE], F32, tag="m1")
                xr = sp.tile([128, FREE], F32, tag="xr")
                hh = sp.tile([128, FREE], F32, tag="hh")
                ww = sp.tile([128, FREE], F32, tag="ww")
                Ee = sp.tile([128, FREE], F32, tag="Ee")
                u1 = sp.tile([128, FREE], F32, tag="u1")
                f1 = sp.tile([128, FREE], F32, tag="f1")
                t2 = sp.tile([128, FREE], F32, tag="t2")
                dd = sp.tile([128, FREE], F32, tag="dd")
                xi = sp.tile([128, FREE], F32, tag="xi")
                h2 = sp.tile([128, FREE], F32, tag="h2")
                w2_ = sp.tile([128, FREE], F32, tag="w2_")
                E2 = sp.tile([128, FREE], F32, tag="E2")
                u2 = sp.tile([128, FREE], F32, tag="u2")
                f2 = sp.tile([128, FREE], F32, tag="f2")

                rr1 = sp.tile([128, FREE], F32, tag="rr1")
                rr2 = sp.tile([128, FREE], F32, tag="rr2")
                nc.gpsimd.tensor_tensor(t1[:], rp_s, rrp_t[:], Alu.mult)         # rel
                if store:
                    nc.scalar.activation(rel_scan[:, s * FREE:(s + 1) * FREE], t1[:], Act.Copy)
                nc.gpsimd.tensor_tensor(rl[:], rrp_t[:], t1[:], Alu.subtract)    # released
                nc.gpsimd.tensor_tensor(m1[:], CP12T[:], ip_t[:], Alu.mult)
                nc.vector.tensor_tensor(xr[:], rl[:], m1[:], Alu.add)            # x_r
                nc.vector.scalar_tensor_tensor(hh[:], xr[:], -1.0, HRT[:], Alu.mult, Alu.add)
                nc.vector.tensor_tensor(ww[:], xr[:], hh[:], Alu.min)
                nc.scalar.activation(rr1[:], ww[:], Act.Relu, bias=1.0, scale=-1.0)
                nc.scalar.activation(Ee[:], rr1[:], Act.Exp, bias=0.0, scale=-1.0)
                nc.vector.tensor_tensor(u1[:], xr[:], Ee[:], Alu.max)
                nc.vector.scalar_tensor_tensor(f1[:], Ee[:], -1.0, HRT[:], Alu.mult, Alu.add)
                nc.vector.tensor_tensor(rrp_t[:], u1[:], f1[:], Alu.min)         # rrp'
                nc.gpsimd.tensor_tensor(t2[:], rl[:], rrp_t[:], Alu.subtract)    # -transfer
                nc.gpsimd.tensor_tensor(dd[:], ip_t[:], t2[:], Alu.add)
                nc.gpsimd.tensor_tensor(xi[:], dd[:], CP01T[:], Alu.add)
                nc.vector.scalar_tensor_tensor(h2[:], dd[:], -1.0, HICPT[:], Alu.mult, Alu.add)
                nc.vector.tensor_tensor(w2_[:], xi[:], h2[:], Alu.min)
                nc.scalar.activation(rr2[:], w2_[:], Act.Relu, bias=1.0, scale=-1.0)
                nc.scalar.activation(E2[:], rr2[:], Act.Exp, bias=0.0, scale=-1.0)
                nc.vector.tensor_tensor(u2[:], xi[:], E2[:], Alu.max)
                nc.vector.scalar_tensor_tensor(f2[:], E2[:], -1.0, HIT[:], Alu.mult, Alu.add)
                nc.vector.tensor_tensor(ip_t[:], u2[:], f2[:], Alu.min)          # ip'

            for s in range(CLEN - W_WARM, CLEN):
                step(s, rrp, ipt, store=False)

            # shift warmup-final states to next chunk; crude-fill chunk 0 of each slot-col
            nc.vector.tensor_copy(rrp2[:], HRT[:])
            nc.vector.tensor_copy(ipt2[:], HIT[:])
            if FSLOT > 1:
                src = rrp[:].rearrange("p (c f) -> p c f", f=FSLOT)
                dst = rrp2[:].rearrange("p (c f) -> p c f", f=FSLOT)
                nc.vector.tensor_copy(dst[:, :, 1:FSLOT], src[:, :, 0:FSLOT - 1])
                srci = ipt[:].rearrange("p (c f) -> p c f", f=FSLOT)
                dsti = ipt2[:].rearrange("p (c f) -> p c f", f=FSLOT)
                nc.vector.tensor_copy(dsti[:, :, 1:FSLOT], srci[:, :, 0:FSLOT - 1])
                nc.sync.dma_start(dst[1:128, :, 0:1], src[0:127, :, FSLOT - 1:FSLOT])
                nc.sync.dma_start(dsti[1:128, :, 0:1], srci[0:127, :, FSLOT - 1:FSLOT])
            else:
                nc.sync.dma_start(rrp2[1:128, :], rrp[0:127, :])
                nc.sync.dma_start(ipt2[1:128, :], ipt[0:127, :])

            for s in range(CLEN):
                step(s, rrp2, ipt2, store=True)

            # ---- phase 3: transpose back, conv2, affine, out ----
            ob_blocks = []
            n0 = 0
            while n0 < U:
                nn = min(512, U - n0)
                ob_blocks.append((n0, nn))
                n0 += nn
            for c in range(NUM_CELLS):
                rlt = mp.tile([128, NJ], F32, tag="rlt")
                for f8 in range(FSLOT):
                    tpp = ptp.tile([128, 128], F16, tag="tp16")
                    base = c * FSLOT + f8
                    nc.tensor.transpose(tpp[:], rel_scan[:, base:base + (CLEN - 1) * FREE + 1:FREE], idf16[:])
                    nc.vector.tensor_copy(rlt[:, f8::FSLOT], tpp[:])
                o2f = sp.tile([128, U], F32, tag="o2f")
                for (b0, bn) in ob_blocks:
                    ps = pcv.tile([128, 512], F32, tag="pconv")
                    nc.tensor.matmul(ps[:, 0:bn], g1[:], rlt[:, b0 + 1:b0 + 1 + bn], start=True, stop=False)
                    nc.tensor.matmul(ps[:, 0:bn], g2[:], rlt[:, b0 + 2:b0 + 2 + bn], start=False, stop=True)
                    nc.vector.tensor_copy(o2f[:, b0:b0 + bn], ps[:, 0:bn])
                oct_ = sp.tile([128, U], F32, tag="oct")
                for b in range(U // 128):
                    tpp = ptp.tile([128, 128], F32, tag="tp")
                    nc.tensor.transpose(tpp[:], o2f[:, b::U // 128], idf32[:])
                    nc.scalar.activation(oct_[:, b * 128:(b + 1) * 128], tpp[:], Act.Identity,
                                         bias=FBT[:, c:c + 1], scale=FNT[:, c:c + 1])
                nc.sync.dma_start(out_e[c].rearrange("(p u) -> p u", u=U), oct_[:])

    nc.compile()
    prog = _Prog()
    prog.nc = nc
    prog.SP = SP
    prog.NJ = NJ
    prog.FSLOT = FSLOT
    prog.FREE = FREE
    prog.XS_LEN = XS_LEN
    _PROG_CACHE[SP] = prog
    return prog


def make_inputs_for_core(k, SP, prog, xp_ext, params):
    """Build the in_map for core k."""
    XS_LEN = prog.XS_LEN
    base = 512 + k * SP
    xs = xp_ext[base:base + XS_LEN]
    m = dict(params)
    m["xs"] = np.ascontiguousarray(xs)
    return m


def host_prep(inputs, SP, prog, ncores=NCORES):
    x = _f32(inputs["x"])
    # extended padded stimulus: [pad 690 of x[0]] + x + tail pad of x[-1]
    tail = prog.XS_LEN + 512 + (ncores - 1) * SP - (PADDING + len(x)) + 8
    xp_ext = np.concatenate([
        np.full(PADDING, x[0], np.float32), x,
        np.full(max(tail, 8), x[-1], np.float32)])

    w = _compute_kernel_np(inputs["log_kernel_speed"], inputs["cell_types"])  # (C,K0)
    W1 = np.zeros((NUM_CELLS, 128, 128), np.float32)
    W2 = np.zeros((NUM_CELLS, 128, 128), np.float32)
    for c in range(NUM_CELLS):
        for p in range(128):
            for m_ in range(128):
                d1 = p - m_
                if 0 <= d1 < K0:
                    W1[c, p, m_] = w[c, d1]
                d2 = 128 + p - m_
                if 0 <= d2 < K0:
                    W2[c, p, m_] = w[c, d2]
    g = _f32(inputs["iglusnfr_kernel"]).reshape(-1)  # (K1,)
    G1 = np.zeros((128, 128), np.float32)
    G2 = np.zeros((128, 128), np.float32)
    for p in range(128):
        for m_ in range(128):
            d1 = p - m_
            if 0 <= d1 < K1:
                G1[p, m_] = g[d1]
            d2 = 128 + p - m_
            if 0 <= d2 < K1:
                G2[p, m_] = g[d2]

    Hr = np.exp(_smooth_clamp_np(_f32(inputs["log_release_pool_capacity"]), 1e6)).astype(np.float32)
    Hi = np.exp(_smooth_clamp_np(_f32(inputs["log_intermediate_pool_capacity"]), 1e6)).astype(np.float32)
    cp01 = np.exp(_f32(inputs["log_change_prob01"])).astype(np.float32)
    cp12 = np.exp(_f32(inputs["log_change_prob12"])).astype(np.float32)
    FSLOT = prog.FSLOT
    cc = np.zeros((5, NUM_CELLS * FSLOT), np.float32)
    for i, v in enumerate([Hr, Hi, cp12, cp01, (Hi - cp01).astype(np.float32)]):
        cc[i] = np.repeat(v, FSLOT)
    slope = np.exp(_f32(inputs["log_sigmoid_slope"])).astype(np.float32)
    off = _f32(inputs["sigmoid_offset"])
    sg = np.stack([slope, (-(slope * off)).astype(np.float32)])
    fs = np.exp(_f32(inputs["log_final_scale"])).astype(np.float32)
    fb = _f32(inputs["final_bias"])
    fn = np.stack([fs, fb])

    params = dict(
        w1=W1, w2=W2, g1=G1, g2=G2,
        idf32=np.eye(128, dtype=np.float32),
        idf16=np.eye(128, dtype=np.float16),
        cc=cc, sg=sg, fn=fn)
    return xp_ext, params


def kernel(**inputs):
    from concourse.bass_utils import run_bass_kernel_spmd
    SP = D // NCORES
    prog = build_program(SP)
    xp_ext, params = host_prep(inputs, SP, prog)
    in_maps = [make_inputs_for_core(k, SP, prog, xp_ext, params) for k in range(NCORES)]
    res = run_bass_kernel_spmd(prog.nc, in_maps, list(range(NCORES)))
    out = np.concatenate([res.results[k]["out"] for k in range(NCORES)], axis=1)
    return out.astype(np.float32)
